# revision 59
# baseline (speedup 1.0000x reference)
"""Trainium2 Bass kernel for nn_DCTModel: bilinear x8 upsample + RGB->YCbCr +
8x8 block DCT + channel selection, fused into two dense matmuls per plane.

Math: the whole reference pipeline is linear in x (all affine offsets only
shift the DC coefficient, which is excluded from the output), so

    out[b, r, (u,i), (v,j)] = (Th @ Xhat[b,r] @ Tw^T)[(u,i), (v,j)]

with Xhat[b,r] = sum_c 127.5*RGB2YCBCR[r,c] * x[b,c]  (112x112),
Th = C @ Ah (DCT-harmonics x bilinear-upsample, [8*112, 112]) with the
orthonormal alpha(u)/2 scale folded in; Tw identical. 54 of the 64 (u,v)
DCT channels are kept.

On-chip per (b, r) plane:
  mix (DVE)            Xhat = sum_c M2[r,c] x[b,c]          -> fp16 [112,112]
  matmul 1 (PE, fp16)  A1t[w,(u,i)] = Xhat^T @ ThT          -> PSUM -> fp16
  matmul 2 (PE, fp16)  Yu[i,(v,j)]  = A1t[:,u-slice]^T @ TwT -> PSUM f32
  copies (DVE+ACT)     PSUM -> per-u staging tile (fp32)
  DMA (sync/HWDGE)     staging -> out[b, ch0:ch1] per u

fp16 keeps |values| < ~1e3 (well in range); measured end-to-end rel err
~4e-4 vs the fp32 reference.

Sharding: pure data parallel, batch 16 -> 2 per core across 8 cores.
"""

import numpy as np

L = 112
SIZE = 8
BS_PER_CORE = 2
N_CORES = 8
SUB_CHANNELS = {0, 1, 2, 3, 4, 5, 8, 9, 16, 24}

RGB2YCBCR = np.asarray(
    [[0.299, 0.587, 0.114],
     [-0.168736, -0.331264, 0.5],
     [0.5, -0.418688, -0.081312]], np.float32)

# per-u: first selected v (selected v's are the contiguous range [V_LO[u], 8))
V_LO = []
M_START = []
_m = 0
for _u in range(SIZE):
    _sel = [_v for _v in range(SIZE) if _u * SIZE + _v not in SUB_CHANNELS]
    assert _sel == list(range(_sel[0], SIZE))
    V_LO.append(_sel[0])
    M_START.append(_m)
    _m += len(_sel)
assert _m == 54


def _build_consts():
    """ThT[h', u*112+i] = alpha(u)/2 * sum_x h[x,u] * Ah[8i+x, h']  (fp16)."""
    Lo = L * SIZE
    src = np.arange(Lo) * (L - 1) / (Lo - 1)
    i0 = np.minimum(np.floor(src).astype(np.int64), L - 2)
    w = (src - i0).astype(np.float32)
    A = np.zeros((Lo, L), np.float32)
    A[np.arange(Lo), i0] = 1.0 - w
    A[np.arange(Lo), i0 + 1] = w

    xg = np.arange(SIZE) + 0.5
    ug = np.arange(SIZE)
    h = np.cos(np.outer(xg, ug) * np.pi / SIZE).astype(np.float32)
    alpha = np.ones(SIZE, np.float32)
    alpha[0] = 1.0 / np.sqrt(2.0)

    Ab = A.reshape(L, SIZE, L)  # [i, x, h']
    Th = np.einsum('xu,ixh->uih', h, Ab).astype(np.float32)
    Th = Th * (alpha / 2.0)[:, None, None]
    ThT = np.ascontiguousarray(Th.transpose(2, 0, 1).reshape(L, SIZE * L))
    # ThT padded to 128-col u-blocks (zeros in cols [u*128+112, (u+1)*128)):
    # matmul2's stationary operand is a 128-wide slice of matmul1's output,
    # which gives the PE full-array utilization + automatic fast weight load.
    ThTp = np.zeros((L, SIZE * 128), np.float16)
    for u in range(SIZE):
        ThTp[:, u * 128:u * 128 + L] = ThT[:, u * L:(u + 1) * L]
    TwT = ThT.astype(np.float16)
    return ThTp, TwT


_CACHE = {}


def _build_program():
    import concourse.bacc as bacc
    import concourse.mybir as mybir
    import concourse.tile as tile

    f32 = mybir.dt.float32
    f16 = mybir.dt.float16
    mult = mybir.AluOpType.mult
    add = mybir.AluOpType.add

    M2 = (127.5 * RGB2YCBCR).astype(np.float32)

    nc = bacc.Bacc(
        "TRN2",
        target_bir_lowering=False,
        debug=False,
        enable_asserts=False,
        num_devices=N_CORES,
    )
    # DRAM layouts are chosen for DMA efficiency; the host transposes:
    #   x fed as [b, h, c, w]  -> SBUF [h, (c w)] loads are fully contiguous
    #   out written [b, i, ch, j] -> per-partition (ch j) runs are contiguous
    #     (the [b, ch, i, j] layout forces isolated 448B HBM writes, which
    #     fall below the 512B SDMA line-rate threshold -> RMW at ~55% eff)
    # out is stored as fp16 and upcast to f32 on the host: HBM write traffic
    # halves (16.3 -> 8.1 MB/core; the kernel is store-bandwidth-bound) and
    # fp16 rounding adds only ~5e-4 relative error (|out| < 3e3 << fp16 max).
    x_d = nc.dram_tensor("x", [BS_PER_CORE, L, 3, L], f32, kind="ExternalInput").ap()
    tht_d = nc.dram_tensor("tht", [L, SIZE * 128], f16,
                           kind="ExternalInput").ap()
    twt_d = nc.dram_tensor("twt", [L, SIZE * L], f16, kind="ExternalInput").ap()
    out_d = nc.dram_tensor(
        "out", [BS_PER_CORE, L, 162, L], f16, kind="ExternalOutput"
    ).ap()

    with tile.TileContext(nc) as tc:
        with tc.tile_pool(name="consts", bufs=1) as cpool, \
             tc.tile_pool(name="xin", bufs=2) as xpool, \
             tc.tile_pool(name="mix", bufs=3) as mpool, \
             tc.tile_pool(name="work", bufs=2) as wpool, \
             tc.tile_pool(name="outb", bufs=6) as opool, \
             tc.tile_pool(name="ps", bufs=8, space="PSUM") as ppool:
            # x for b=0 first: it heads the critical path. Split the b=0
            # load per color channel so the mix can start as soon as each
            # channel lands rather than waiting on the full 150KB tile.
            # All input loads go on the Scalar HWDGE ring (the Sync ring is
            # reserved for the output stream), ordered by first consumption:
            # x channels for b=0, tht slices per r, twt, then b=1.
            xbs = []
            for b in range(BS_PER_CORE):
                xb = xpool.tile([L, 3, L], f32, name=f"xb{b}", tag="xb")
                xbs.append(xb)
            tht = cpool.tile([L, SIZE * 128], f16, name="tht_sb")
            # channel 1 heads the critical path (first mix op): split it
            # across both rings so its halves land in parallel.
            nc.scalar.dma_start(xbs[0][0:56, 1, :], x_d[0, 0:56, 1, :])
            nc.sync.dma_start(xbs[0][56:L, 1, :], x_d[0, 56:L, 1, :])
            nc.scalar.dma_start(tht[:], tht_d[:])
            nc.scalar.dma_start(xbs[0][:, 0, :], x_d[0, :, 0, :])
            nc.scalar.dma_start(xbs[0][:, 2, :], x_d[0, :, 2, :])
            twt = cpool.tile([L, SIZE * L], f16, name="twt_sb")
            nc.scalar.dma_start(twt[:], twt_d[:])
            nc.scalar.dma_start(xbs[1][:], x_d[1])

            # PE_HAM warm-up: the PE clock sits at 1.2GHz (K=4/8) until
            # ~3.4us of sustained activity, and the real matmul stream only
            # starts once x lands (~10us, after the ~7us framework preamble
            # + load latency).  Run a train of dependency-free 128-col
            # matmuls on a zeroed dummy tile during that dead window so the
            # array is at 2.4GHz when matmul1 issues.  They cycle through
            # the psum pool before any real allocation and have no readers.
            warm_w = cpool.tile([128, 128], f16, name="warm_w")
            nc.gpsimd.memset(warm_w[:], 0)
            for i in range(36):
                wps = ppool.tile([128, 128], f32, name=f"warm{i}", tag="ps")
                nc.tensor.matmul(wps[:], lhsT=warm_w[:], rhs=warm_w[:],
                                 start=True, stop=True)

            # Alternate PSUM->SBUF copies between DVE and ACT. ACT also issues
            # half the output DMAs (second HWDGE ring), so DVE gets more cols.
            copy_cols = {"v": 0, "s": 0}

            def psum_copy(dst, src, ncols):
                if copy_cols["v"] * 1.1 <= copy_cols["s"]:
                    nc.vector.tensor_copy(dst, src)
                    copy_cols["v"] += ncols
                else:
                    nc.scalar.copy(dst, src)
                    copy_cols["s"] += ncols

            n_dma = [0]

            def emit_mix(b, r):
                """YCbCr mix for plane (b, r) -> fp16 xh tile (DVE).

                Emitted ~2 planes ahead of the plane's matmul2 stream so it
                sits in the DVE queue BEFORE earlier planes' PSUM->SBUF
                staging copies (engine program order is serial — a late mix
                gates the next matmul1 and bubbles the store stream).
                """
                xb = xbs[b]
                tmp = mpool.tile([L, L], f32, name=f"tmp{b}{r}", tag="tmp")
                xh = mpool.tile([L, L], f16, name=f"xh{b}{r}", tag="xh")
                nc.vector.tensor_scalar_mul(tmp[:], xb[:, 1, :],
                                            float(M2[r, 1]))
                nc.vector.scalar_tensor_tensor(
                    tmp[:], xb[:, 0, :], float(M2[r, 0]), tmp[:], mult, add)
                nc.vector.scalar_tensor_tensor(
                    xh[:], xb[:, 2, :], float(M2[r, 2]), tmp[:], mult, add)
                return xh

            def emit_stage1(k, b, r, xh):
                """matmul1 for plane k; returns the a1t tile."""
                # matmul 1: A1t[w, (u,i)] = Xhat^T @ ThTp -> [112, 1024],
                # in 128-col u-blocks (pad cols are zeros baked into ThTp).
                a1t = wpool.tile([L, SIZE * 128], f16, name=f"a1t{b}{r}",
                                 tag="a1t")
                for half in range(2):
                    ps = ppool.tile([L, 512], f32, name=f"psA{b}{r}{half}",
                                    tag="ps")
                    nc.tensor.matmul(
                        ps[:],
                        lhsT=xh[:],
                        rhs=tht[:, 512 * half:512 * (half + 1)],
                        start=True, stop=True)
                    psum_copy(a1t[:, 512 * half:512 * (half + 1)], ps[:], 512)
                return a1t

            def emit_stage2(b, r, a1t, first=False, last=False):
                """matmul2 + staging copies + output DMAs for one plane.

                The whole plane stages into one [112, 54*112] f32 tile whose
                per-partition line is exactly DRAM row i's contiguous
                channel block [r*54:(r+1)*54) x j — descriptors are multi-KB
                fully-contiguous runs (vs 448B descriptors for the
                [b, ch, i, j] layout, which sit below the 512B SDMA
                line-rate threshold).  Store granularity: one DMA per
                u-block on the first plane (starts the store stream earlier
                while the pipeline fills) and on the last plane (small final
                drain quantum); one DMA per 2-u group (~0.7MB) in between.
                """
                ust = opool.tile([L, 54 * L], f16, name=f"ust{b}{r}",
                                 tag="ust")
                per_u = first or last
                dma_after = set(range(SIZE)) if per_u else {1, 3, 5, 7}
                grp_c0 = [0]

                def flush(u):
                    # per_u mode flushes exactly u's channels; group mode
                    # flushes everything staged since the last flush.
                    m0 = M_START[u] if per_u else grp_c0[0]
                    c0 = r * 54 + m0
                    c1 = r * 54 + M_START[u] + (SIZE - V_LO[u])
                    nsel = c1 - c0
                    ucol0 = m0 * L
                    # All store DMAs issue from the Sync ring: Sync is
                    # otherwise pure semaphore-waiting, while every DMA
                    # instruction on ACT (~0.6us each) steals copy time.
                    nc.sync.dma_start(
                        out_d[b, :, c0:c1, :],
                        ust[:, ucol0:ucol0 + nsel * L]
                        .rearrange("p (m j) -> p m j", j=L))
                    grp_c0[0] = c1 - r * 54

                # Last plane runs u descending so the final drain quantum is
                # the smallest u-block (u0: 2 channels, 50KB).
                u_order = reversed(range(SIZE)) if last else range(SIZE)
                for u in u_order:
                    # 128-wide stationary slice -> full PE array + FWL;
                    # rows 112..127 of the PSUM result are zeros.
                    lhs_u = a1t[:, u * 128:(u + 1) * 128]
                    v0 = V_LO[u]
                    col = M_START[u] * L
                    ncols = (SIZE - v0) * L
                    t0 = v0 * L
                    if ncols <= 512:
                        ps0 = ppool.tile([128, ncols], f32,
                                         name=f"ps0_{b}{r}{u}", tag="ps")
                        nc.tensor.matmul(
                            ps0[:], lhsT=lhs_u,
                            rhs=twt[:, t0:t0 + ncols],
                            start=True, stop=True)
                        psum_copy(ust[:, col:col + ncols], ps0[0:L, :],
                                  ncols)
                    else:
                        c = ncols // 2
                        ps0 = ppool.tile([128, c], f32,
                                         name=f"ps0_{b}{r}{u}", tag="ps")
                        nc.tensor.matmul(
                            ps0[:], lhsT=lhs_u,
                            rhs=twt[:, t0:t0 + c],
                            start=True, stop=True)
                        psum_copy(ust[:, col:col + c], ps0[0:L, :], c)
                        ps1 = ppool.tile([128, c], f32,
                                         name=f"ps1_{b}{r}{u}", tag="ps")
                        nc.tensor.matmul(
                            ps1[:], lhsT=lhs_u,
                            rhs=twt[:, t0 + c:t0 + ncols],
                            start=True, stop=True)
                        psum_copy(ust[:, col + c:col + ncols], ps1[0:L, :],
                                  c)

                    if u in dma_after:
                        flush(u)

            # Software-pipeline the 6 planes: plane k+1's matmul1 is emitted
            # before plane k's matmul2 stream (PE never idles across the a1t
            # copy boundary).
            planes = [(b, r) for b in range(BS_PER_CORE) for r in range(3)]
            xhs = [emit_mix(*planes[0]), emit_mix(*planes[1])]
            prev = None
            for k, (b, r) in enumerate(planes):
                a1t = emit_stage1(k, b, r, xhs[k])
                if k + 2 < len(planes):
                    xhs.append(emit_mix(*planes[k + 2]))
                if prev is not None:
                    emit_stage2(prev[0], prev[1], prev[2], first=(k == 1))
                prev = (b, r, a1t)
            emit_stage2(prev[0], prev[1], prev[2], last=True)

    nc.compile()
    return nc


def kernel(x: np.ndarray) -> np.ndarray:
    from concourse import bass_utils

    x = np.asarray(x, np.float32)
    assert x.shape == (BS_PER_CORE * N_CORES, 3, L, L)
    # device-friendly layouts (see _build_program): x as [b, h, c, w],
    # out comes back [b, i, ch, j]
    xt = np.ascontiguousarray(x.transpose(0, 2, 1, 3))

    if "nc" not in _CACHE:
        _CACHE["nc"] = _build_program()
        _CACHE["consts"] = _build_consts()
    nc = _CACHE["nc"]
    ThT, TwT = _CACHE["consts"]

    in_maps = [
        {"x": xt[c * BS_PER_CORE:(c + 1) * BS_PER_CORE], "tht": ThT, "twt": TwT}
        for c in range(N_CORES)
    ]
    res = bass_utils.run_bass_kernel_spmd(nc, in_maps, core_ids=list(range(N_CORES)))
    out = np.concatenate(
        [res.results[c]["out"].transpose(0, 2, 1, 3) for c in range(N_CORES)],
        axis=0)
    return np.ascontiguousarray(out.astype(np.float32))



# revision 61
# speedup vs baseline: 1.0350x; 1.0350x over previous
"""Trainium2 Bass kernel for nn_DCTModel: bilinear x8 upsample + RGB->YCbCr +
8x8 block DCT + channel selection, fused into two dense matmuls per plane.

Math: the whole reference pipeline is linear in x (all affine offsets only
shift the DC coefficient, which is excluded from the output), so

    out[b, r, (u,i), (v,j)] = (Th @ Xhat[b,r] @ Tw^T)[(u,i), (v,j)]

with Xhat[b,r] = sum_c 127.5*RGB2YCBCR[r,c] * x[b,c]  (112x112),
Th = C @ Ah (DCT-harmonics x bilinear-upsample, [8*112, 112]) with the
orthonormal alpha(u)/2 scale folded in; Tw identical. 54 of the 64 (u,v)
DCT channels are kept.

On-chip per (b, r) plane:
  mix (DVE)            Xhat = sum_c M2[r,c] x[b,c]          -> fp16 [112,112]
  matmul 1 (PE, fp16)  A1t[w,(u,i)] = Xhat^T @ ThT          -> PSUM -> fp16
  matmul 2 (PE, fp16)  Yu[i,(v,j)]  = A1t[:,u-slice]^T @ TwT -> PSUM f32
  copies (DVE+ACT)     PSUM -> fp16 plane staging tile [112, 54*112]
  DMA (Sync HWDGE)     staging -> out fp16 [b, i, ch, j]; host upcasts

Performance notes (measured on trn2, 8 cores, ~48us vs 89us baseline):
  - The kernel is store-bound: out f32 would be 16.3MB/core vs a ~360GB/s
    per-core DMA ceiling.  Storing fp16 (host upcasts) halves that; fp16
    rounding adds ~5e-5 to the relative error (total ~5e-4 << 2e-2).
  - DRAM layouts are transposed on the host so every DMA descriptor is a
    multi-KB contiguous run (sub-512B HBM writes pay a ~2x RMW penalty):
    x as [b, h, c, w], out as [b, i, ch, j].
  - PE_HAM warm-up matmuls run during the dead window between the
    framework preamble and the x-load landing, so the PE array is at
    2.4GHz (not the cold 1.2GHz) when the real matmul stream starts.
  - PSUM->SBUF copies are the bottleneck engine work (fp32-source copies
    run in 1x mode on both DVE and ACT); they are split DVE:ACT ~ 1:1.1
    and the mixes are hoisted 2 planes ahead in the DVE queue.
  - All store DMAs issue from the otherwise-idle Sync ring; inputs load
    on the Scalar ring.  Stores flush per-u on the first plane (early
    stream start), per-2u mid-planes, and per-u in descending u order on
    the last plane so the final drain quantum is the smallest block.

Sharding: pure data parallel, batch 16 -> 2 per core across 8 cores.
"""

import numpy as np

L = 112
SIZE = 8
BS_PER_CORE = 2
N_CORES = 8
SUB_CHANNELS = {0, 1, 2, 3, 4, 5, 8, 9, 16, 24}

RGB2YCBCR = np.asarray(
    [[0.299, 0.587, 0.114],
     [-0.168736, -0.331264, 0.5],
     [0.5, -0.418688, -0.081312]], np.float32)

# per-u: first selected v (selected v's are the contiguous range [V_LO[u], 8))
V_LO = []
M_START = []
_m = 0
for _u in range(SIZE):
    _sel = [_v for _v in range(SIZE) if _u * SIZE + _v not in SUB_CHANNELS]
    assert _sel == list(range(_sel[0], SIZE))
    V_LO.append(_sel[0])
    M_START.append(_m)
    _m += len(_sel)
assert _m == 54


def _build_consts():
    """ThT[h', u*112+i] = alpha(u)/2 * sum_x h[x,u] * Ah[8i+x, h']  (fp16)."""
    Lo = L * SIZE
    src = np.arange(Lo) * (L - 1) / (Lo - 1)
    i0 = np.minimum(np.floor(src).astype(np.int64), L - 2)
    w = (src - i0).astype(np.float32)
    A = np.zeros((Lo, L), np.float32)
    A[np.arange(Lo), i0] = 1.0 - w
    A[np.arange(Lo), i0 + 1] = w

    xg = np.arange(SIZE) + 0.5
    ug = np.arange(SIZE)
    h = np.cos(np.outer(xg, ug) * np.pi / SIZE).astype(np.float32)
    alpha = np.ones(SIZE, np.float32)
    alpha[0] = 1.0 / np.sqrt(2.0)

    Ab = A.reshape(L, SIZE, L)  # [i, x, h']
    Th = np.einsum('xu,ixh->uih', h, Ab).astype(np.float32)
    Th = Th * (alpha / 2.0)[:, None, None]
    ThT = np.ascontiguousarray(Th.transpose(2, 0, 1).reshape(L, SIZE * L))
    # ThT padded to 128-col u-blocks (zeros in cols [u*128+112, (u+1)*128)):
    # matmul2's stationary operand is a 128-wide slice of matmul1's output,
    # which gives the PE full-array utilization + automatic fast weight load.
    ThTp = np.zeros((L, SIZE * 128), np.float16)
    for u in range(SIZE):
        ThTp[:, u * 128:u * 128 + L] = ThT[:, u * L:(u + 1) * L]
    TwT = ThT.astype(np.float16)
    return ThTp, TwT


_CACHE = {}


def _build_program():
    import concourse.bacc as bacc
    import concourse.mybir as mybir
    import concourse.tile as tile

    f32 = mybir.dt.float32
    f16 = mybir.dt.float16
    mult = mybir.AluOpType.mult
    add = mybir.AluOpType.add

    M2 = (127.5 * RGB2YCBCR).astype(np.float32)

    nc = bacc.Bacc(
        "TRN2",
        target_bir_lowering=False,
        debug=False,
        enable_asserts=False,
        num_devices=N_CORES,
    )
    # DRAM layouts are chosen for DMA efficiency; the host transposes:
    #   x fed as [b, h, c, w]  -> SBUF [h, (c w)] loads are fully contiguous
    #   out written [b, i, ch, j] -> per-partition (ch j) runs are contiguous
    #     (the [b, ch, i, j] layout forces isolated 448B HBM writes, which
    #     fall below the 512B SDMA line-rate threshold -> RMW at ~55% eff)
    # out is stored as fp16 and upcast to f32 on the host: HBM write traffic
    # halves (16.3 -> 8.1 MB/core; the kernel is store-bandwidth-bound) and
    # fp16 rounding adds only ~5e-4 relative error (|out| < 3e3 << fp16 max).
    x_d = nc.dram_tensor("x", [BS_PER_CORE, L, 3, L], f32, kind="ExternalInput").ap()
    tht_d = nc.dram_tensor("tht", [L, SIZE * 128], f16,
                           kind="ExternalInput").ap()
    twt_d = nc.dram_tensor("twt", [L, SIZE * L], f16, kind="ExternalInput").ap()
    out_d = nc.dram_tensor(
        "out", [BS_PER_CORE, L, 162, L], f16, kind="ExternalOutput"
    ).ap()

    with tile.TileContext(nc) as tc:
        with tc.tile_pool(name="consts", bufs=1) as cpool, \
             tc.tile_pool(name="xin", bufs=2) as xpool, \
             tc.tile_pool(name="mix", bufs=3) as mpool, \
             tc.tile_pool(name="work", bufs=2) as wpool, \
             tc.tile_pool(name="outb", bufs=6) as opool, \
             tc.tile_pool(name="ps", bufs=8, space="PSUM") as ppool:
            # x for b=0 first: it heads the critical path. Split the b=0
            # load per color channel so the mix can start as soon as each
            # channel lands rather than waiting on the full 150KB tile.
            # All input loads go on the Scalar HWDGE ring (the Sync ring is
            # reserved for the output stream), ordered by first consumption:
            # x channels for b=0, tht slices per r, twt, then b=1.
            xbs = []
            for b in range(BS_PER_CORE):
                xb = xpool.tile([L, 3, L], f32, name=f"xb{b}", tag="xb")
                xbs.append(xb)
            tht = cpool.tile([L, SIZE * 128], f16, name="tht_sb")
            nc.scalar.dma_start(xbs[0][:, 1, :], x_d[0, :, 1, :])
            nc.scalar.dma_start(tht[:], tht_d[:])
            nc.scalar.dma_start(xbs[0][:, 0, :], x_d[0, :, 0, :])
            nc.scalar.dma_start(xbs[0][:, 2, :], x_d[0, :, 2, :])
            twt = cpool.tile([L, SIZE * L], f16, name="twt_sb")
            nc.scalar.dma_start(twt[:], twt_d[:])
            nc.scalar.dma_start(xbs[1][:], x_d[1])

            # PE_HAM warm-up: the PE clock sits at 1.2GHz (K=4/8) until
            # ~3.4us of sustained activity, and the real matmul stream only
            # starts once x lands (~10us, after the ~7us framework preamble
            # + load latency).  Run a train of dependency-free 128-col
            # matmuls on a zeroed dummy tile during that dead window so the
            # array is at 2.4GHz when matmul1 issues.  They cycle through
            # the psum pool before any real allocation and have no readers.
            warm_w = cpool.tile([128, 128], f16, name="warm_w")
            nc.gpsimd.memset(warm_w[:], 0)
            for i in range(36):
                wps = ppool.tile([128, 128], f32, name=f"warm{i}", tag="ps")
                nc.tensor.matmul(wps[:], lhsT=warm_w[:], rhs=warm_w[:],
                                 start=True, stop=True)

            # Alternate PSUM->SBUF copies between DVE and ACT. ACT also issues
            # half the output DMAs (second HWDGE ring), so DVE gets more cols.
            copy_cols = {"v": 0, "s": 0}

            def psum_copy(dst, src, ncols):
                if copy_cols["v"] * 1.1 <= copy_cols["s"]:
                    nc.vector.tensor_copy(dst, src)
                    copy_cols["v"] += ncols
                else:
                    nc.scalar.copy(dst, src)
                    copy_cols["s"] += ncols

            n_dma = [0]

            def emit_mix(b, r):
                """YCbCr mix for plane (b, r) -> fp16 xh tile (DVE).

                Emitted ~2 planes ahead of the plane's matmul2 stream so it
                sits in the DVE queue BEFORE earlier planes' PSUM->SBUF
                staging copies (engine program order is serial — a late mix
                gates the next matmul1 and bubbles the store stream).
                """
                xb = xbs[b]
                tmp = mpool.tile([L, L], f32, name=f"tmp{b}{r}", tag="tmp")
                xh = mpool.tile([L, L], f16, name=f"xh{b}{r}", tag="xh")
                nc.vector.tensor_scalar_mul(tmp[:], xb[:, 1, :],
                                            float(M2[r, 1]))
                nc.vector.scalar_tensor_tensor(
                    tmp[:], xb[:, 0, :], float(M2[r, 0]), tmp[:], mult, add)
                nc.vector.scalar_tensor_tensor(
                    xh[:], xb[:, 2, :], float(M2[r, 2]), tmp[:], mult, add)
                return xh

            def emit_stage1(k, b, r, xh):
                """matmul1 for plane k; returns the a1t tile."""
                # matmul 1: A1t[w, (u,i)] = Xhat^T @ ThTp -> [112, 1024],
                # in 128-col u-blocks (pad cols are zeros baked into ThTp).
                a1t = wpool.tile([L, SIZE * 128], f16, name=f"a1t{b}{r}",
                                 tag="a1t")
                for half in range(2):
                    ps = ppool.tile([L, 512], f32, name=f"psA{b}{r}{half}",
                                    tag="ps")
                    nc.tensor.matmul(
                        ps[:],
                        lhsT=xh[:],
                        rhs=tht[:, 512 * half:512 * (half + 1)],
                        start=True, stop=True)
                    psum_copy(a1t[:, 512 * half:512 * (half + 1)], ps[:], 512)
                return a1t

            def emit_stage2(b, r, a1t, first=False, last=False):
                """matmul2 + staging copies + output DMAs for one plane.

                The whole plane stages into one [112, 54*112] f32 tile whose
                per-partition line is exactly DRAM row i's contiguous
                channel block [r*54:(r+1)*54) x j — descriptors are multi-KB
                fully-contiguous runs (vs 448B descriptors for the
                [b, ch, i, j] layout, which sit below the 512B SDMA
                line-rate threshold).  Store granularity: one DMA per
                u-block on the first plane (starts the store stream earlier
                while the pipeline fills) and on the last plane (small final
                drain quantum); one DMA per 2-u group (~0.7MB) in between.
                """
                ust = opool.tile([L, 54 * L], f16, name=f"ust{b}{r}",
                                 tag="ust")
                per_u = first or last
                dma_after = set(range(SIZE)) if per_u else {1, 3, 5, 7}
                grp_c0 = [0]

                def flush(u):
                    # per_u mode flushes exactly u's channels; group mode
                    # flushes everything staged since the last flush.
                    m0 = M_START[u] if per_u else grp_c0[0]
                    c0 = r * 54 + m0
                    c1 = r * 54 + M_START[u] + (SIZE - V_LO[u])
                    nsel = c1 - c0
                    ucol0 = m0 * L
                    # All store DMAs issue from the Sync ring: Sync is
                    # otherwise pure semaphore-waiting, while every DMA
                    # instruction on ACT (~0.6us each) steals copy time.
                    nc.sync.dma_start(
                        out_d[b, :, c0:c1, :],
                        ust[:, ucol0:ucol0 + nsel * L]
                        .rearrange("p (m j) -> p m j", j=L))
                    grp_c0[0] = c1 - r * 54

                # Last plane runs u descending so the final drain quantum is
                # the smallest u-block (u0: 2 channels, 50KB).
                u_order = reversed(range(SIZE)) if last else range(SIZE)
                for u in u_order:
                    # 128-wide stationary slice -> full PE array + FWL;
                    # rows 112..127 of the PSUM result are zeros.
                    lhs_u = a1t[:, u * 128:(u + 1) * 128]
                    v0 = V_LO[u]
                    col = M_START[u] * L
                    ncols = (SIZE - v0) * L
                    t0 = v0 * L
                    if ncols <= 512:
                        ps0 = ppool.tile([128, ncols], f32,
                                         name=f"ps0_{b}{r}{u}", tag="ps")
                        nc.tensor.matmul(
                            ps0[:], lhsT=lhs_u,
                            rhs=twt[:, t0:t0 + ncols],
                            start=True, stop=True)
                        psum_copy(ust[:, col:col + ncols], ps0[0:L, :],
                                  ncols)
                    else:
                        c = ncols // 2
                        ps0 = ppool.tile([128, c], f32,
                                         name=f"ps0_{b}{r}{u}", tag="ps")
                        nc.tensor.matmul(
                            ps0[:], lhsT=lhs_u,
                            rhs=twt[:, t0:t0 + c],
                            start=True, stop=True)
                        psum_copy(ust[:, col:col + c], ps0[0:L, :], c)
                        ps1 = ppool.tile([128, c], f32,
                                         name=f"ps1_{b}{r}{u}", tag="ps")
                        nc.tensor.matmul(
                            ps1[:], lhsT=lhs_u,
                            rhs=twt[:, t0 + c:t0 + ncols],
                            start=True, stop=True)
                        psum_copy(ust[:, col + c:col + ncols], ps1[0:L, :],
                                  c)

                    if u in dma_after:
                        flush(u)

            # Software-pipeline the 6 planes: plane k+1's matmul1 is emitted
            # before plane k's matmul2 stream (PE never idles across the a1t
            # copy boundary).
            planes = [(b, r) for b in range(BS_PER_CORE) for r in range(3)]
            xhs = [emit_mix(*planes[0]), emit_mix(*planes[1])]
            prev = None
            for k, (b, r) in enumerate(planes):
                a1t = emit_stage1(k, b, r, xhs[k])
                if k + 2 < len(planes):
                    xhs.append(emit_mix(*planes[k + 2]))
                if prev is not None:
                    emit_stage2(prev[0], prev[1], prev[2], first=(k == 1))
                prev = (b, r, a1t)
            emit_stage2(prev[0], prev[1], prev[2], last=True)

    nc.compile()
    return nc


def kernel(x: np.ndarray) -> np.ndarray:
    from concourse import bass_utils

    x = np.asarray(x, np.float32)
    assert x.shape == (BS_PER_CORE * N_CORES, 3, L, L)
    # device-friendly layouts (see _build_program): x as [b, h, c, w],
    # out comes back [b, i, ch, j]
    xt = np.ascontiguousarray(x.transpose(0, 2, 1, 3))

    if "nc" not in _CACHE:
        _CACHE["nc"] = _build_program()
        _CACHE["consts"] = _build_consts()
    nc = _CACHE["nc"]
    ThT, TwT = _CACHE["consts"]

    in_maps = [
        {"x": xt[c * BS_PER_CORE:(c + 1) * BS_PER_CORE], "tht": ThT, "twt": TwT}
        for c in range(N_CORES)
    ]
    res = bass_utils.run_bass_kernel_spmd(nc, in_maps, core_ids=list(range(N_CORES)))
    out = np.concatenate(
        [res.results[c]["out"].transpose(0, 2, 1, 3) for c in range(N_CORES)],
        axis=0)
    return np.ascontiguousarray(out.astype(np.float32))



# revision 66
# speedup vs baseline: 1.0544x; 1.0187x over previous
"""Trainium2 Bass kernel for nn_DCTModel: bilinear x8 upsample + RGB->YCbCr +
8x8 block DCT + channel selection, fused into two dense matmuls per plane.

Math: the whole reference pipeline is linear in x (all affine offsets only
shift the DC coefficient, which is excluded from the output), so

    out[b, r, (u,i), (v,j)] = (Th @ Xhat[b,r] @ Tw^T)[(u,i), (v,j)]

with Xhat[b,r] = sum_c 127.5*RGB2YCBCR[r,c] * x[b,c]  (112x112),
Th = C @ Ah (DCT-harmonics x bilinear-upsample, [8*112, 112]) with the
orthonormal alpha(u)/2 scale folded in; Tw identical. 54 of the 64 (u,v)
DCT channels are kept.

On-chip per (b, r) plane:
  mix (DVE)            Xhat = sum_c M2[r,c] x[b,c]          -> fp16 [112,112]
  matmul 1 (PE, fp16)  A1t[w,(u,i)] = Xhat^T @ ThT          -> PSUM -> fp16
  matmul 2 (PE, fp16)  Yu[i,(v,j)]  = A1t[:,u-slice]^T @ TwT -> PSUM f32
  copies (DVE+ACT)     PSUM -> fp16 plane staging tile [112, 54*112]
  DMA (Sync HWDGE)     staging -> out fp16 [b, i, ch, j]; host upcasts

Performance notes (measured on trn2, 8 cores, ~48us vs 89us baseline):
  - The kernel is store-bound: out f32 would be 16.3MB/core vs a ~360GB/s
    per-core DMA ceiling.  Storing fp16 (host upcasts) halves that; fp16
    rounding adds ~5e-5 to the relative error (total ~5e-4 << 2e-2).
  - DRAM layouts are transposed on the host so every DMA descriptor is a
    multi-KB contiguous run (sub-512B HBM writes pay a ~2x RMW penalty):
    x as [b, h, c, w], out as [b, i, ch, j].
  - PE_HAM warm-up matmuls run during the dead window between the
    framework preamble and the x-load landing, so the PE array is at
    2.4GHz (not the cold 1.2GHz) when the real matmul stream starts.
  - PSUM->SBUF copies are the bottleneck engine work (fp32-source copies
    run in 1x mode on both DVE and ACT); they are split DVE:ACT ~ 1:1.1
    and the mixes are hoisted 2 planes ahead in the DVE queue.
  - All store DMAs issue from the otherwise-idle Sync ring; inputs load
    on the Scalar ring.  Stores flush per-u on the first plane (early
    stream start), per-2u mid-planes, and per-u in descending u order on
    the last plane so the final drain quantum is the smallest block.

Sharding: pure data parallel, batch 16 -> 2 per core across 8 cores.
"""

import numpy as np

L = 112
SIZE = 8
BS_PER_CORE = 2
N_CORES = 8
SUB_CHANNELS = {0, 1, 2, 3, 4, 5, 8, 9, 16, 24}

RGB2YCBCR = np.asarray(
    [[0.299, 0.587, 0.114],
     [-0.168736, -0.331264, 0.5],
     [0.5, -0.418688, -0.081312]], np.float32)

# per-u: first selected v (selected v's are the contiguous range [V_LO[u], 8))
V_LO = []
M_START = []
_m = 0
for _u in range(SIZE):
    _sel = [_v for _v in range(SIZE) if _u * SIZE + _v not in SUB_CHANNELS]
    assert _sel == list(range(_sel[0], SIZE))
    V_LO.append(_sel[0])
    M_START.append(_m)
    _m += len(_sel)
assert _m == 54


def _build_consts():
    """ThT[h', u*112+i] = alpha(u)/2 * sum_x h[x,u] * Ah[8i+x, h']  (fp16)."""
    Lo = L * SIZE
    src = np.arange(Lo) * (L - 1) / (Lo - 1)
    i0 = np.minimum(np.floor(src).astype(np.int64), L - 2)
    w = (src - i0).astype(np.float32)
    A = np.zeros((Lo, L), np.float32)
    A[np.arange(Lo), i0] = 1.0 - w
    A[np.arange(Lo), i0 + 1] = w

    xg = np.arange(SIZE) + 0.5
    ug = np.arange(SIZE)
    h = np.cos(np.outer(xg, ug) * np.pi / SIZE).astype(np.float32)
    alpha = np.ones(SIZE, np.float32)
    alpha[0] = 1.0 / np.sqrt(2.0)

    Ab = A.reshape(L, SIZE, L)  # [i, x, h']
    Th = np.einsum('xu,ixh->uih', h, Ab).astype(np.float32)
    Th = Th * (alpha / 2.0)[:, None, None]
    ThT = np.ascontiguousarray(Th.transpose(2, 0, 1).reshape(L, SIZE * L))
    # ThT padded to 128-col u-blocks (zeros in cols [u*128+112, (u+1)*128)):
    # matmul2's stationary operand is a 128-wide slice of matmul1's output,
    # which gives the PE full-array utilization + automatic fast weight load.
    ThTp = np.zeros((L, SIZE * 128), np.float16)
    for u in range(SIZE):
        ThTp[:, u * 128:u * 128 + L] = ThT[:, u * L:(u + 1) * L]
    TwT = ThT.astype(np.float16)
    return ThTp, TwT


_CACHE = {}


def _build_program():
    import concourse.bacc as bacc
    import concourse.mybir as mybir
    import concourse.tile as tile

    f32 = mybir.dt.float32
    f16 = mybir.dt.float16
    mult = mybir.AluOpType.mult
    add = mybir.AluOpType.add

    M2 = (127.5 * RGB2YCBCR).astype(np.float32)

    nc = bacc.Bacc(
        "TRN2",
        target_bir_lowering=False,
        debug=False,
        enable_asserts=False,
        num_devices=N_CORES,
    )
    # DRAM layouts are chosen for DMA efficiency; the host transposes:
    #   x fed as [b, h, c, w]  -> SBUF [h, (c w)] loads are fully contiguous
    #   out written [b, i, ch, j] -> per-partition (ch j) runs are contiguous
    #     (the [b, ch, i, j] layout forces isolated 448B HBM writes, which
    #     fall below the 512B SDMA line-rate threshold -> RMW at ~55% eff)
    # out is stored as fp16 and upcast to f32 on the host: HBM write traffic
    # halves (16.3 -> 8.1 MB/core; the kernel is store-bandwidth-bound) and
    # fp16 rounding adds only ~5e-4 relative error (|out| < 3e3 << fp16 max).
    # x is fed as fp16 (host casts): halves the load bytes and lets the mix
    # run in the DVE's 2x 16-bit mode (fp16 keeps x's ~5e-4 precision).
    x_d = nc.dram_tensor("x", [BS_PER_CORE, L, 3, L], f16, kind="ExternalInput").ap()
    tht_d = nc.dram_tensor("tht", [L, SIZE * 128], f16,
                           kind="ExternalInput").ap()
    twt_d = nc.dram_tensor("twt", [L, SIZE * L], f16, kind="ExternalInput").ap()
    out_d = nc.dram_tensor(
        "out", [BS_PER_CORE, L, 162, L], f16, kind="ExternalOutput"
    ).ap()

    with tile.TileContext(nc) as tc:
        with tc.tile_pool(name="consts", bufs=1) as cpool, \
             tc.tile_pool(name="xin", bufs=2) as xpool, \
             tc.tile_pool(name="mix", bufs=3) as mpool, \
             tc.tile_pool(name="work", bufs=2) as wpool, \
             tc.tile_pool(name="outb", bufs=6) as opool, \
             tc.tile_pool(name="ps", bufs=8, space="PSUM") as ppool:
            # x for b=0 first: it heads the critical path. Split the b=0
            # load per color channel so the mix can start as soon as each
            # channel lands rather than waiting on the full 150KB tile.
            # All input loads go on the Scalar HWDGE ring (the Sync ring is
            # reserved for the output stream), ordered by first consumption:
            # x channels for b=0, tht slices per r, twt, then b=1.
            xbs = []
            for b in range(BS_PER_CORE):
                xb = xpool.tile([L, 3, L], f16, name=f"xb{b}", tag="xb")
                xbs.append(xb)
            tht = cpool.tile([L, SIZE * 128], f16, name="tht_sb")
            nc.scalar.dma_start(xbs[0][:, 1, :], x_d[0, :, 1, :])
            nc.scalar.dma_start(tht[:], tht_d[:])
            nc.scalar.dma_start(xbs[0][:, 0, :], x_d[0, :, 0, :])
            nc.scalar.dma_start(xbs[0][:, 2, :], x_d[0, :, 2, :])
            twt = cpool.tile([L, SIZE * L], f16, name="twt_sb")
            nc.scalar.dma_start(twt[:], twt_d[:])
            nc.scalar.dma_start(xbs[1][:], x_d[1])

            # PE_HAM warm-up: the PE clock sits at 1.2GHz (K=4/8) until
            # ~3.4us of sustained activity, and the real matmul stream only
            # starts once x lands (~10us, after the ~7us framework preamble
            # + load latency).  Run a train of dependency-free 128-col
            # matmuls on a zeroed dummy tile during that dead window so the
            # array is at 2.4GHz when matmul1 issues.  They cycle through
            # the psum pool before any real allocation and have no readers.
            warm_w = cpool.tile([128, 128], f16, name="warm_w")
            nc.gpsimd.memset(warm_w[:], 0)
            for i in range(36):
                wps = ppool.tile([128, 128], f32, name=f"warm{i}", tag="ps")
                nc.tensor.matmul(wps[:], lhsT=warm_w[:], rhs=warm_w[:],
                                 start=True, stop=True)

            # Alternate PSUM->SBUF copies between DVE and ACT. ACT also issues
            # half the output DMAs (second HWDGE ring), so DVE gets more cols.
            copy_cols = {"v": 0, "s": 0}

            def psum_copy(dst, src, ncols):
                if copy_cols["v"] * 1.1 <= copy_cols["s"]:
                    nc.vector.tensor_copy(dst, src)
                    copy_cols["v"] += ncols
                else:
                    nc.scalar.copy(dst, src)
                    copy_cols["s"] += ncols

            n_dma = [0]

            def emit_mix(b, r):
                """YCbCr mix for plane (b, r) -> fp16 xh tile (DVE).

                Emitted ~2 planes ahead of the plane's matmul2 stream so it
                sits in the DVE queue BEFORE earlier planes' PSUM->SBUF
                staging copies (engine program order is serial — a late mix
                gates the next matmul1 and bubbles the store stream).
                """
                xb = xbs[b]
                tmp = mpool.tile([L, L], f16, name=f"tmp{b}{r}", tag="tmp")
                xh = mpool.tile([L, L], f16, name=f"xh{b}{r}", tag="xh")
                nc.vector.tensor_scalar_mul(tmp[:], xb[:, 1, :],
                                            float(M2[r, 1]))
                nc.vector.scalar_tensor_tensor(
                    tmp[:], xb[:, 0, :], float(M2[r, 0]), tmp[:], mult, add)
                nc.vector.scalar_tensor_tensor(
                    xh[:], xb[:, 2, :], float(M2[r, 2]), tmp[:], mult, add)
                return xh

            def emit_stage1(k, b, r, xh):
                """matmul1 for plane k; returns the a1t tile."""
                # matmul 1: A1t[w, (u,i)] = Xhat^T @ ThTp -> [112, 1024],
                # in 128-col u-blocks (pad cols are zeros baked into ThTp).
                a1t = wpool.tile([L, SIZE * 128], f16, name=f"a1t{b}{r}",
                                 tag="a1t")
                for half in range(2):
                    ps = ppool.tile([L, 512], f32, name=f"psA{b}{r}{half}",
                                    tag="ps")
                    nc.tensor.matmul(
                        ps[:],
                        lhsT=xh[:],
                        rhs=tht[:, 512 * half:512 * (half + 1)],
                        start=True, stop=True)
                    # Copy only the 112 real columns of each 128-wide
                    # u-block; the 16 pad columns feed matmul2 output rows
                    # 112-127, which are never read — garbage is fine.
                    dst = a1t[:, 512 * half:512 * (half + 1)].rearrange(
                        "p (u k) -> p u k", k=128)[:, :, 0:L]
                    src = ps[:].rearrange(
                        "p (u k) -> p u k", k=128)[:, :, 0:L]
                    psum_copy(dst, src, 448)
                return a1t

            def emit_stage2(b, r, a1t, first=False, last=False):
                """matmul2 + staging copies + output DMAs for one plane.

                The whole plane stages into one [112, 54*112] f32 tile whose
                per-partition line is exactly DRAM row i's contiguous
                channel block [r*54:(r+1)*54) x j — descriptors are multi-KB
                fully-contiguous runs (vs 448B descriptors for the
                [b, ch, i, j] layout, which sit below the 512B SDMA
                line-rate threshold).  Store granularity: one DMA per
                u-block on the first plane (starts the store stream earlier
                while the pipeline fills) and on the last plane (small final
                drain quantum); one DMA per 2-u group (~0.7MB) in between.
                """
                ust = opool.tile([L, 54 * L], f16, name=f"ust{b}{r}",
                                 tag="ust")
                per_u = first or last
                dma_after = set(range(SIZE)) if per_u else {1, 3, 5, 7}
                grp_c0 = [0]

                def flush(u):
                    # per_u mode flushes exactly u's channels; group mode
                    # flushes everything staged since the last flush.
                    m0 = M_START[u] if per_u else grp_c0[0]
                    c0 = r * 54 + m0
                    c1 = r * 54 + M_START[u] + (SIZE - V_LO[u])
                    nsel = c1 - c0
                    ucol0 = m0 * L
                    # All store DMAs issue from the Sync ring: Sync is
                    # otherwise pure semaphore-waiting, while every DMA
                    # instruction on ACT (~0.6us each) steals copy time.
                    nc.sync.dma_start(
                        out_d[b, :, c0:c1, :],
                        ust[:, ucol0:ucol0 + nsel * L]
                        .rearrange("p (m j) -> p m j", j=L))
                    grp_c0[0] = c1 - r * 54

                # Last plane runs u descending so the final drain quantum is
                # the smallest u-block (u0: 2 channels, 50KB).
                u_order = reversed(range(SIZE)) if last else range(SIZE)
                for u in u_order:
                    # 128-wide stationary slice -> full PE array + FWL;
                    # rows 112..127 of the PSUM result are zeros.
                    lhs_u = a1t[:, u * 128:(u + 1) * 128]
                    v0 = V_LO[u]
                    col = M_START[u] * L
                    ncols = (SIZE - v0) * L
                    t0 = v0 * L
                    if ncols <= 512:
                        ps0 = ppool.tile([128, ncols], f32,
                                         name=f"ps0_{b}{r}{u}", tag="ps")
                        nc.tensor.matmul(
                            ps0[:], lhsT=lhs_u,
                            rhs=twt[:, t0:t0 + ncols],
                            start=True, stop=True)
                        psum_copy(ust[:, col:col + ncols], ps0[0:L, :],
                                  ncols)
                    else:
                        c = ncols // 2
                        ps0 = ppool.tile([128, c], f32,
                                         name=f"ps0_{b}{r}{u}", tag="ps")
                        nc.tensor.matmul(
                            ps0[:], lhsT=lhs_u,
                            rhs=twt[:, t0:t0 + c],
                            start=True, stop=True)
                        psum_copy(ust[:, col:col + c], ps0[0:L, :], c)
                        ps1 = ppool.tile([128, c], f32,
                                         name=f"ps1_{b}{r}{u}", tag="ps")
                        nc.tensor.matmul(
                            ps1[:], lhsT=lhs_u,
                            rhs=twt[:, t0 + c:t0 + ncols],
                            start=True, stop=True)
                        psum_copy(ust[:, col + c:col + ncols], ps1[0:L, :],
                                  c)

                    if u in dma_after:
                        flush(u)

            # Software-pipeline the 6 planes: plane k+1's matmul1 is emitted
            # before plane k's matmul2 stream (PE never idles across the a1t
            # copy boundary).
            planes = [(b, r) for b in range(BS_PER_CORE) for r in range(3)]
            xhs = [emit_mix(*planes[0]), emit_mix(*planes[1])]
            prev = None
            for k, (b, r) in enumerate(planes):
                a1t = emit_stage1(k, b, r, xhs[k])
                if k + 2 < len(planes):
                    xhs.append(emit_mix(*planes[k + 2]))
                if prev is not None:
                    emit_stage2(prev[0], prev[1], prev[2], first=(k == 1))
                prev = (b, r, a1t)
            emit_stage2(prev[0], prev[1], prev[2], last=True)

    nc.compile()
    return nc


def kernel(x: np.ndarray) -> np.ndarray:
    from concourse import bass_utils

    x = np.asarray(x, np.float32)
    assert x.shape == (BS_PER_CORE * N_CORES, 3, L, L)
    # device-friendly layouts (see _build_program): x as [b, h, c, w] fp16,
    # out comes back [b, i, ch, j] fp16
    xt = np.ascontiguousarray(x.transpose(0, 2, 1, 3).astype(np.float16))

    if "nc" not in _CACHE:
        _CACHE["nc"] = _build_program()
        _CACHE["consts"] = _build_consts()
    nc = _CACHE["nc"]
    ThT, TwT = _CACHE["consts"]

    in_maps = [
        {"x": xt[c * BS_PER_CORE:(c + 1) * BS_PER_CORE], "tht": ThT, "twt": TwT}
        for c in range(N_CORES)
    ]
    res = bass_utils.run_bass_kernel_spmd(nc, in_maps, core_ids=list(range(N_CORES)))
    out = np.concatenate(
        [res.results[c]["out"].transpose(0, 2, 1, 3) for c in range(N_CORES)],
        axis=0)
    return np.ascontiguousarray(out.astype(np.float32))

